# revision 45
# baseline (speedup 1.0000x reference)
"""MultiHeadCoAttention Trainium2 kernel.

Math (per batch b, H=16 heads of d=64, L1=L2=1024, M=1024):
  K1 = ctx1 @ Wk1.T + bk1; V1 = ctx1 @ Wv1.T + bv1  (D1=512)
  K2 = ctx2 @ Wk2.T + bk2; V2 = ctx2 @ Wv2.T + bv2  (D2=1024)
  scores_h[q,k] = K2_h[q,:] . K1_h[k,:]
  dist12 = softmax over q (axis=2 in [B,H,q,k] -> per (h,k) over q)
  ctx12_h[q,:] = sum_k (E_h[q,k]/S2_h[k]) V1_h[k,:]   with E=exp(scores), S2_h[k]=sum_q E_h[q,k]
  dist21 = softmax over h: D_h[q,k] = E_h[q,k]/S1[q,k], S1[q,k]=sum_h E_h[q,k]
  ctx21_h[k,:] = sum_q D_h[q,k] V2_h[q,:]
  out12[q,:] = ctx12[q,:] @ Wf12.T + bf12 ; out21[k,:] = ctx21[k,:] @ Wf21.T + bf21

Restructured for HW: no max-subtraction (scores sigma~2.3, exp safe in fp32);
1/S2 folded into V1 rows; E stored bf16; S1 accumulated fp32 on DVE;
scores computed twice (k-major pass 1, q-major pass 2) on PE.
Sharding: data-parallel, one batch per core (8 cores).
"""

import numpy as np
from contextlib import ExitStack

L = 1024      # L1 == L2
D1 = 512
D2 = 1024
M = 1024
H = 16
DPH = 64
B = 8
P = 128
PC = L // P   # 8 partition-chunks per 1024-dim

_CACHE = {}


def _build():
    import concourse.bass as bass
    import concourse.bacc as bacc
    import concourse.mybir as mybir
    import concourse.tile as tile
    from concourse.masks import make_identity

    f32 = mybir.dt.float32
    f32r = mybir.dt.float32r
    bf16 = mybir.dt.bfloat16
    EXP = mybir.ActivationFunctionType.Exp
    IDENT = mybir.ActivationFunctionType.Identity

    nc = bacc.Bacc("TRN2")

    c1_d = nc.dram_tensor("ctx1", [L, D1], f32, kind="ExternalInput").ap()
    c2_d = nc.dram_tensor("ctx2", [L, D2], f32, kind="ExternalInput").ap()
    wk1_d = nc.dram_tensor("wk1", [M, D1], f32, kind="ExternalInput").ap()
    wv1_d = nc.dram_tensor("wv1", [M, D1], f32, kind="ExternalInput").ap()
    wk2_d = nc.dram_tensor("wk2", [M, D2], f32, kind="ExternalInput").ap()
    wv2_d = nc.dram_tensor("wv2", [M, D2], f32, kind="ExternalInput").ap()
    wf12_d = nc.dram_tensor("wf12", [M, M], f32, kind="ExternalInput").ap()
    wf21_d = nc.dram_tensor("wf21", [M, M], f32, kind="ExternalInput").ap()
    bk1_d = nc.dram_tensor("bk1", [M], f32, kind="ExternalInput").ap()
    bv1_d = nc.dram_tensor("bv1", [M], f32, kind="ExternalInput").ap()
    bk2_d = nc.dram_tensor("bk2", [M], f32, kind="ExternalInput").ap()
    bv2_d = nc.dram_tensor("bv2", [M], f32, kind="ExternalInput").ap()
    bf12_d = nc.dram_tensor("bf12", [M], f32, kind="ExternalInput").ap()
    bf21_d = nc.dram_tensor("bf21", [M], f32, kind="ExternalInput").ap()
    o21_d = nc.dram_tensor("out21", [L, M], f32, kind="ExternalOutput").ap()
    o12_d = nc.dram_tensor("out12", [L, M], f32, kind="ExternalOutput").ap()

    def bcast_ap(vec_ap, parts=P):
        return bass.AP(tensor=vec_ap.tensor, offset=vec_ap.offset,
                       ap=[[0, parts]] + list(vec_ap.ap))

    with tile.TileContext(nc) as tc, ExitStack() as top:
        dma = nc.default_dma_engine

        persist = top.enter_context(tc.tile_pool(name="persist", bufs=1))
        small = top.enter_context(tc.tile_pool(name="small", bufs=1))
        wfw = top.enter_context(tc.tile_pool(name="wfw", bufs=1))
        wfb = top.enter_context(tc.tile_pool(name="wfb", bufs=1))

        ident = small.tile([P, P], f32)
        make_identity(nc, ident[:])

        # ---- persistent SBUF tensors (whole kernel) ----
        k1t = persist.tile([P, PC, L], bf16)   # K1T[m,l]: [p, mc, l] 2MB
        k2t = persist.tile([P, PC, L], bf16)   # 2MB
        v1 = persist.tile([P, PC, M], bf16)    # V1 token-major [p, lc, m] 2MB
        v2 = persist.tile([P, PC, M], bf16)    # 2MB

        # K-biases as per-partition columns [p, mc]
        bkcol = small.tile([P, 2 * PC], f32)   # [:, 0:8]=bk1, [:, 8:16]=bk2
        s2 = small.tile([P, PC], f32)
        invs2 = small.tile([P, PC], f32)

        # ---------- helpers ----------
        def transpose_1024(src, src_cchunks, dst, psum_pool, dst_off=0,
                           evac=None):
            """src [P, AC, BC*128] -> dst [P, BC, dst_off + AC*128] transposed.

            src holds X[a, b] as [p, ac, b] with a=ac*128+p;
            dst receives XT[b, a] as [p, bc, dst_off + a].
            """
            if evac is None:
                evac = nc.scalar.copy
            AC = src_cchunks
            BC = src.shape[2] // P
            for bc in range(BC):
                for grp in range((AC + 3) // 4):
                    lo = grp * 4
                    hi = min(lo + 4, AC)
                    pt = psum_pool.tile([P, 512], f32)
                    for ac in range(lo, hi):
                        nc.tensor.matmul(
                            pt[:, (ac - lo) * P:(ac - lo + 1) * P],
                            src[:, ac, bc * P:(bc + 1) * P],
                            ident[:],
                            is_transpose=True,
                        )
                    evac(dst[:, bc, dst_off + lo * P:dst_off + hi * P],
                         pt[:, :(hi - lo) * P])

        def load_T(src_d, cols, dst, sbpool, ppool, evac=None):
            """DMA [1024, cols] DRAM tensor and PE-transpose into dst [P, cols//128, 1024]."""
            r = src_d.rearrange("(rc p) c -> p rc c", p=P)
            for ch in range(2):
                sb = sbpool.tile([P, 4, cols], f32)
                dma.dma_start(out=sb[:], in_=r[:, ch * 4:(ch + 1) * 4, :])
                transpose_1024(sb, 4, dst, ppool, dst_off=ch * 512, evac=evac)

        # ---------- phase A/B: loads, transposes, projections ----------
        with ExitStack() as ph:
            ppool = ph.enter_context(tc.tile_pool(name="ps_a", bufs=4, space="PSUM"))
            pproj = ph.enter_context(tc.tile_pool(name="ps_pj", bufs=2, space="PSUM"))
            bias_small = ph.enter_context(tc.tile_pool(name="bs", bufs=1))

            # K-bias columns: [1024] -> [8,128] sbuf -> PE transpose -> [128,8]
            for i, bd in enumerate((bk1_d, bk2_d)):
                brow = bias_small.tile([PC, P], f32)
                dma.dma_start(out=brow[:], in_=bd.rearrange("(a b) -> a b", a=PC))
                pt = ppool.tile([P, 512], f32)
                nc.tensor.matmul(pt[:, 0:PC], brow[:],
                                 ident[0:PC, 0:PC], is_transpose=True)
                nc.scalar.copy(bkcol[:, i * PC:(i + 1) * PC], pt[:, 0:PC])

            # V-bias broadcast tiles
            bv1_bc = bias_small.tile([P, M], f32)
            dma.dma_start(out=bv1_bc[:], in_=bcast_ap(bv1_d))
            bv2_bc = bias_small.tile([P, M], f32)
            dma.dma_start(out=bv2_bc[:], in_=bcast_ap(bv2_d))

            def project(wd, CC, ct, wtpool, sbpool, bias_cols=None, bias_bc=None,
                        out_fm=None, out_tm=None):
                """Load W [M, CC*128], transpose to WT [c, m], then matmul.

                out_fm: feature-major out [p, mc, l] f32 (K-style), lhsT=WT rhs=CT.
                out_tm: token-major out [p, lc, m] bf16 (V-style), lhsT=CT rhs=WT.
                """
                wt = wtpool.tile([P, CC, M], bf16)
                load_T(wd, CC * P, wt, sbpool, ppool)
                for oc in range(PC):
                    for nn in range(2):
                        pt = pproj.tile([P, 512], f32)
                        for cc in range(CC):
                            if out_fm is not None:
                                lhs = wt[:, cc, oc * P:(oc + 1) * P]
                                rhs = ct[:, cc, nn * 512:(nn + 1) * 512]
                            else:
                                lhs = ct[:, cc, oc * P:(oc + 1) * P]
                                rhs = wt[:, cc, nn * 512:(nn + 1) * 512]
                            nc.tensor.matmul(pt[:], lhs, rhs,
                                             start=(cc == 0), stop=(cc == CC - 1))
                        if out_fm is not None:
                            nc.scalar.activation(
                                out_fm[:, oc, nn * 512:(nn + 1) * 512], pt[:],
                                IDENT, bias=bias_cols[:, oc:oc + 1])
                        else:
                            nc.vector.tensor_add(
                                out_tm[:, oc, nn * 512:(nn + 1) * 512], pt[:],
                                bias_bc[:, nn * 512:(nn + 1) * 512])

            with ExitStack() as p1:
                sb1 = p1.enter_context(tc.tile_pool(name="sb1", bufs=2))
                tp1 = p1.enter_context(tc.tile_pool(name="tp1", bufs=1))
                wtp1 = p1.enter_context(tc.tile_pool(name="wtp1", bufs=1))
                c1t = tp1.tile([P, D1 // P, L], bf16)  # [p, cc(4), l] 1MB
                load_T(c1_d, D1, c1t, sb1, ppool)
                project(wk1_d, D1 // P, c1t, wtp1, sb1,
                        bias_cols=bkcol[:, 0:PC], out_fm=k1t)
                project(wv1_d, D1 // P, c1t, wtp1, sb1,
                        bias_bc=bv1_bc, out_tm=v1)

            with ExitStack() as p2:
                sb2 = p2.enter_context(tc.tile_pool(name="sb2", bufs=2))
                tp2 = p2.enter_context(tc.tile_pool(name="tp2", bufs=1))
                wtp2 = p2.enter_context(tc.tile_pool(name="wtp2", bufs=1))
                c2t = tp2.tile([P, PC, L], bf16)       # 2MB
                load_T(c2_d, D2, c2t, sb2, ppool)
                project(wk2_d, D2 // P, c2t, wtp2, sb2,
                        bias_cols=bkcol[:, PC:2 * PC], out_fm=k2t)
                project(wv2_d, D2 // P, c2t, wtp2, sb2,
                        bias_bc=bv2_bc, out_tm=v2)
                # preload Wf12T here: phase A/B is PE-paced, ACT has slack
                wf12wt = wfw.tile([P, PC, M], bf16, tag="wfwt")
                load_T(wf12_d, M, wf12wt, sb2, ppool)
                wf12bb = wfb.tile([P, M], f32, tag="wfbb")
                dma.dma_start(out=wf12bb[:], in_=bcast_ap(bf12_d))

        # per-head K slices: head h -> partitions (h%2)*64.., chunk h//2
        def hd(t, h):
            return t[(h % 2) * DPH:(h % 2 + 1) * DPH, h // 2, :]

        # ---------- out projection helper ----------
        def out_project(wt, bbc, ctxT, out_d, ph):
            """Matmul part only; wt/bbc must be preloaded."""
            pout = ph.enter_context(tc.tile_pool(name="ps_o", bufs=2, space="PSUM"))
            opool = ph.enter_context(tc.tile_pool(name="ob", bufs=3))
            od = out_d.rearrange("(lc p) m -> p lc m", p=P)
            for lc in range(PC):
                for nn in range(2):
                    pt = pout.tile([P, 512], f32)
                    for cc in range(PC):
                        nc.tensor.matmul(
                            pt[:], ctxT[:, cc, lc * P:(lc + 1) * P],
                            wt[:, cc, nn * 512:(nn + 1) * 512],
                            start=(cc == 0), stop=(cc == PC - 1))
                    ot = opool.tile([P, 512], f32)
                    nc.vector.tensor_add(ot[:], pt[:],
                                         bbc[:, nn * 512:(nn + 1) * 512])
                    dma.dma_start(out=od[:, lc, nn * 512:(nn + 1) * 512], in_=ot[:])

        with ExitStack() as rest:
            apool = rest.enter_context(tc.tile_pool(name="attn", bufs=1))
            rq = apool.tile([P, PC, L], bf16)      # R[q,k] bf16: [p, qc, k] 2MB
            ctx12t = apool.tile([P, PC, L], bf16)  # CTX12T[m,q] 2MB

            # ---------- pass 1: k-major. ET, S2, V1', ctx12T, S1T ----------
            with ExitStack() as ph:
                s1pool = ph.enter_context(tc.tile_pool(name="s1", bufs=1))
                etpool = ph.enter_context(tc.tile_pool(name="et", bufs=2))
                psc = ph.enter_context(tc.tile_pool(name="ps_sc", bufs=3, space="PSUM"))
                pctx = ph.enter_context(tc.tile_pool(name="ps_cx", bufs=2, space="PSUM"))
                s1t = s1pool.tile([P, PC, L], f32)   # S1T[k,q]: [p, kc, q] 4MB

                for h in range(H):
                    k1h = hd(k1t, h)   # [64, 1024] (d, k)
                    k2h = hd(k2t, h)   # [64, 1024] (d, q)
                    et = etpool.tile([P, PC, L], bf16)   # ET[k,q] 2MB
                    # scoresT (2 matmuls fill a 2-bank PSUM tile) + exp + S2
                    for kc in range(PC):
                        pt = psc.tile([P, L], f32)
                        for qn in range(2):
                            nc.tensor.matmul(
                                pt[:, qn * 512:(qn + 1) * 512],
                                k1h[:, kc * P:(kc + 1) * P],
                                k2h[:, qn * 512:(qn + 1) * 512],
                                start=True, stop=True)
                        nc.scalar.activation(et[:, kc, :], pt[:], EXP,
                                             accum_out=s2[:, kc:kc + 1])
                    # invS2; V1_h *= invS2 (per-partition scale)
                    nc.vector.reciprocal_approx_fast(invs2[:], s2[:])
                    for kc in range(PC):
                        nc.vector.tensor_scalar_mul(
                            v1[:, kc, h * DPH:(h + 1) * DPH],
                            v1[:, kc, h * DPH:(h + 1) * DPH],
                            invs2[:, kc:kc + 1])
                    # ctx12T_h[d,q] = sum_k V1'_h[k,d] ET[k,q]
                    for qn in range(2):
                        pt = pctx.tile([DPH, 512], f32)
                        for kc in range(PC):
                            nc.tensor.matmul(
                                pt[:], v1[:, kc, h * DPH:(h + 1) * DPH],
                                et[:, kc, qn * 512:(qn + 1) * 512],
                                start=(kc == 0), stop=(kc == PC - 1))
                        nc.vector.tensor_copy(
                            hd(ctx12t, h)[:, qn * 512:(qn + 1) * 512], pt[:])
                    # S1T accumulation (fp32 += bf16): kc 0-5 on DVE, 6-7 on
                    # gpsimd (idle engine; one big contiguous op per head)
                    if h == 0:
                        nc.vector.tensor_copy(s1t[:, 0:6, :], et[:, 0:6, :])
                        nc.gpsimd.tensor_copy(s1t[:, 6:8, :], et[:, 6:8, :])
                    else:
                        nc.vector.tensor_add(s1t[:, 0:6, :], s1t[:, 0:6, :],
                                             et[:, 0:6, :])
                        nc.gpsimd.tensor_add(s1t[:, 6:8, :], s1t[:, 6:8, :],
                                             et[:, 6:8, :])

                # ---------- phase D: R[q,k] = transpose(1/S1T) ----------
                nc.vector.reciprocal_approx_fast(s1t[:, :, :], s1t[:, :, :])
                transpose_1024(s1t, PC, rq, psc)

            # ctx21t allocated only after s1t is freed (SBUF pressure)
            c21pool = rest.enter_context(tc.tile_pool(name="c21", bufs=1))
            ctx21t = c21pool.tile([P, PC, L], bf16)  # CTX21T[m,k] 2MB

            # ---------- out12 projection (wt preloaded mid-pass-1) ----------
            with ExitStack() as ph:
                out_project(wf12wt, wf12bb, ctx12t, o12_d, ph)

            # ---------- pass 2: q-major. E, D=E*R, ctx21T ----------
            with ExitStack() as ph:
                epool = ph.enter_context(tc.tile_pool(name="e2", bufs=2))
                wfsb = ph.enter_context(tc.tile_pool(name="wfsb", bufs=1))
                psc = ph.enter_context(tc.tile_pool(name="ps_s2", bufs=2, space="PSUM"))
                pctx = ph.enter_context(tc.tile_pool(name="ps_c2", bufs=2, space="PSUM"))
                wfp2 = ph.enter_context(tc.tile_pool(name="ps_wf2", bufs=2, space="PSUM"))

                for h in range(H):
                    if h == 8:
                        wf21wt = wfw.tile([P, PC, M], bf16, tag="wfwt")
                        load_T(wf21_d, M, wf21wt, wfsb, wfp2,
                               evac=nc.vector.tensor_copy)
                        wf21bb = wfb.tile([P, M], f32, tag="wfbb")
                        dma.dma_start(out=wf21bb[:], in_=bcast_ap(bf21_d))
                    k1h = hd(k1t, h)
                    k2h = hd(k2t, h)
                    e = epool.tile([P, PC, L], bf16)   # E[q,k] 2MB
                    for qc in range(PC):
                        pt = psc.tile([P, L], f32)
                        for kn in range(2):
                            nc.tensor.matmul(
                                pt[:, kn * 512:(kn + 1) * 512],
                                k2h[:, qc * P:(qc + 1) * P],
                                k1h[:, kn * 512:(kn + 1) * 512],
                                start=True, stop=True)
                        nc.scalar.activation(e[:, qc, :], pt[:], EXP)
                        nc.vector.tensor_mul(e[:, qc, :], e[:, qc, :],
                                             rq[:, qc, :])
                    # ctx21T_h[d,k] = sum_q V2_h[q,d] D[q,k]
                    for kn in range(2):
                        pt = pctx.tile([DPH, 512], f32)
                        for qc in range(PC):
                            nc.tensor.matmul(
                                pt[:], v2[:, qc, h * DPH:(h + 1) * DPH],
                                e[:, qc, kn * 512:(kn + 1) * 512],
                                start=(qc == 0), stop=(qc == PC - 1))
                        nc.vector.tensor_copy(
                            hd(ctx21t, h)[:, kn * 512:(kn + 1) * 512], pt[:])

            # ---------- out21 projection ----------
            with ExitStack() as ph:
                out_project(wf21wt, wf21bb, ctx21t, o21_d, ph)

    nc.finalize()
    return nc


def _run(inputs, trace=False, tmpdir=None):
    from concourse import bass_utils
    if "nc" not in _CACHE:
        _CACHE["nc"] = _build()
    nc = _CACHE["nc"]

    shared = {
        "wk1": np.ascontiguousarray(inputs["Wk1"], np.float32),
        "wv1": np.ascontiguousarray(inputs["Wv1"], np.float32),
        "wk2": np.ascontiguousarray(inputs["Wk2"], np.float32),
        "wv2": np.ascontiguousarray(inputs["Wv2"], np.float32),
        "wf12": np.ascontiguousarray(inputs["Wf12"], np.float32),
        "wf21": np.ascontiguousarray(inputs["Wf21"], np.float32),
        "bk1": np.ascontiguousarray(inputs["bk1"], np.float32),
        "bv1": np.ascontiguousarray(inputs["bv1"], np.float32),
        "bk2": np.ascontiguousarray(inputs["bk2"], np.float32),
        "bv2": np.ascontiguousarray(inputs["bv2"], np.float32),
        "bf12": np.ascontiguousarray(inputs["bf12"], np.float32),
        "bf21": np.ascontiguousarray(inputs["bf21"], np.float32),
    }
    in_maps = []
    for b in range(B):
        m = dict(shared)
        m["ctx1"] = np.ascontiguousarray(inputs["ctx_1"][:, b, :], np.float32)
        m["ctx2"] = np.ascontiguousarray(inputs["ctx_2"][:, b, :], np.float32)
        in_maps.append(m)

    res = bass_utils.run_bass_kernel_spmd(
        nc, in_maps, core_ids=list(range(B)), trace=trace, tmpdir=tmpdir)
    out21 = np.stack([np.asarray(res.results[b]["out21"]) for b in range(B)],
                     axis=1).astype(np.float32)
    out12 = np.stack([np.asarray(res.results[b]["out12"]) for b in range(B)],
                     axis=1).astype(np.float32)
    return (out21, out12), res


def kernel(**inputs):
    outs, _ = _run(inputs, trace=False)
    return outs


# revision 48
# speedup vs baseline: 1.1459x; 1.1459x over previous
"""MultiHeadCoAttention Trainium2 kernel.

Math (per batch b, H=16 heads of d=64, L1=L2=1024, M=1024):
  K1 = ctx1 @ Wk1.T + bk1; V1 = ctx1 @ Wv1.T + bv1  (D1=512)
  K2 = ctx2 @ Wk2.T + bk2; V2 = ctx2 @ Wv2.T + bv2  (D2=1024)
  scores_h[q,k] = K2_h[q,:] . K1_h[k,:]
  dist12 = softmax over q (axis=2 in [B,H,q,k] -> per (h,k) over q)
  ctx12_h[q,:] = sum_k (E_h[q,k]/S2_h[k]) V1_h[k,:]   with E=exp(scores), S2_h[k]=sum_q E_h[q,k]
  dist21 = softmax over h: D_h[q,k] = E_h[q,k]/S1[q,k], S1[q,k]=sum_h E_h[q,k]
  ctx21_h[k,:] = sum_q D_h[q,k] V2_h[q,:]
  out12[q,:] = ctx12[q,:] @ Wf12.T + bf12 ; out21[k,:] = ctx21[k,:] @ Wf21.T + bf21

Restructured for HW: no max-subtraction (scores sigma~2.3, exp safe in fp32);
1/S2 folded into V1 rows; E stored bf16; S1 accumulated fp32 on DVE;
scores computed twice (k-major pass 1, q-major pass 2) on PE.
Sharding: data-parallel, one batch per core (8 cores).
"""

import numpy as np
from contextlib import ExitStack

L = 1024      # L1 == L2
D1 = 512
D2 = 1024
M = 1024
H = 16
DPH = 64
B = 8
P = 128
PC = L // P   # 8 partition-chunks per 1024-dim

_CACHE = {}


def _build():
    import concourse.bass as bass
    import concourse.bacc as bacc
    import concourse.mybir as mybir
    import concourse.tile as tile
    from concourse.masks import make_identity

    f32 = mybir.dt.float32
    f32r = mybir.dt.float32r
    bf16 = mybir.dt.bfloat16
    EXP = mybir.ActivationFunctionType.Exp
    IDENT = mybir.ActivationFunctionType.Identity

    nc = bacc.Bacc("TRN2")

    c1_d = nc.dram_tensor("ctx1", [L, D1], f32, kind="ExternalInput").ap()
    c2_d = nc.dram_tensor("ctx2", [L, D2], f32, kind="ExternalInput").ap()
    wk1_d = nc.dram_tensor("wk1", [M, D1], f32, kind="ExternalInput").ap()
    wv1_d = nc.dram_tensor("wv1", [M, D1], f32, kind="ExternalInput").ap()
    wk2_d = nc.dram_tensor("wk2", [M, D2], f32, kind="ExternalInput").ap()
    wv2_d = nc.dram_tensor("wv2", [M, D2], f32, kind="ExternalInput").ap()
    wf12_d = nc.dram_tensor("wf12", [M, M], f32, kind="ExternalInput").ap()
    wf21_d = nc.dram_tensor("wf21", [M, M], f32, kind="ExternalInput").ap()
    bk1_d = nc.dram_tensor("bk1", [M], f32, kind="ExternalInput").ap()
    bv1_d = nc.dram_tensor("bv1", [M], f32, kind="ExternalInput").ap()
    bk2_d = nc.dram_tensor("bk2", [M], f32, kind="ExternalInput").ap()
    bv2_d = nc.dram_tensor("bv2", [M], f32, kind="ExternalInput").ap()
    bf12_d = nc.dram_tensor("bf12", [M], f32, kind="ExternalInput").ap()
    bf21_d = nc.dram_tensor("bf21", [M], f32, kind="ExternalInput").ap()
    o21_d = nc.dram_tensor("out21", [L, M], f32, kind="ExternalOutput").ap()
    o12_d = nc.dram_tensor("out12", [L, M], f32, kind="ExternalOutput").ap()

    def bcast_ap(vec_ap, parts=P):
        return bass.AP(tensor=vec_ap.tensor, offset=vec_ap.offset,
                       ap=[[0, parts]] + list(vec_ap.ap))

    with tile.TileContext(nc) as tc, ExitStack() as top:
        dma = nc.default_dma_engine

        persist = top.enter_context(tc.tile_pool(name="persist", bufs=1))
        small = top.enter_context(tc.tile_pool(name="small", bufs=1))
        wfw = top.enter_context(tc.tile_pool(name="wfw", bufs=1))
        wfb = top.enter_context(tc.tile_pool(name="wfb", bufs=1))

        ident = small.tile([P, P], f32)
        make_identity(nc, ident[:])

        # ---- persistent SBUF tensors (whole kernel) ----
        k1t = persist.tile([P, PC, L], bf16)   # K1T[m,l]: [p, mc, l] 2MB
        k2t = persist.tile([P, PC, L], bf16)   # 2MB
        v1 = persist.tile([P, PC, M], bf16)    # V1 token-major [p, lc, m] 2MB
        v2 = persist.tile([P, PC, M], bf16)    # 2MB

        # K-biases as per-partition columns [p, mc]
        bkcol = small.tile([P, 2 * PC], f32)   # [:, 0:8]=bk1, [:, 8:16]=bk2
        s2 = small.tile([P, PC], f32)
        invs2 = small.tile([P, PC], f32)

        # ---------- helpers ----------
        def transpose_1024(src, src_cchunks, dst, psum_pool, dst_off=0,
                           evac=None):
            """src [P, AC, BC*128] -> dst [P, BC, dst_off + AC*128] transposed.

            src holds X[a, b] as [p, ac, b] with a=ac*128+p;
            dst receives XT[b, a] as [p, bc, dst_off + a].
            """
            if evac is None:
                evac = nc.scalar.copy
            AC = src_cchunks
            BC = src.shape[2] // P
            for bc in range(BC):
                for grp in range((AC + 3) // 4):
                    lo = grp * 4
                    hi = min(lo + 4, AC)
                    pt = psum_pool.tile([P, 512], f32)
                    for ac in range(lo, hi):
                        nc.tensor.matmul(
                            pt[:, (ac - lo) * P:(ac - lo + 1) * P],
                            src[:, ac, bc * P:(bc + 1) * P],
                            ident[:],
                            is_transpose=True,
                        )
                    evac(dst[:, bc, dst_off + lo * P:dst_off + hi * P],
                         pt[:, :(hi - lo) * P])

        def load_T(src_d, cols, dst, sbpool, ppool, evac=None):
            """DMA [1024, cols] DRAM tensor and PE-transpose into dst [P, cols//128, 1024]."""
            r = src_d.rearrange("(rc p) c -> p rc c", p=P)
            for ch in range(2):
                sb = sbpool.tile([P, 4, cols], f32)
                dma.dma_start(out=sb[:], in_=r[:, ch * 4:(ch + 1) * 4, :])
                transpose_1024(sb, 4, dst, ppool, dst_off=ch * 512, evac=evac)

        # ---------- phase A/B: loads, transposes, projections ----------
        with ExitStack() as ph:
            ppool = ph.enter_context(tc.tile_pool(name="ps_a", bufs=4, space="PSUM"))
            pproj = ph.enter_context(tc.tile_pool(name="ps_pj", bufs=2, space="PSUM"))
            bias_small = ph.enter_context(tc.tile_pool(name="bs", bufs=1))

            # K-bias columns: [1024] -> [8,128] sbuf -> PE transpose -> [128,8]
            for i, bd in enumerate((bk1_d, bk2_d)):
                brow = bias_small.tile([PC, P], f32)
                dma.dma_start(out=brow[:], in_=bd.rearrange("(a b) -> a b", a=PC))
                pt = ppool.tile([P, 512], f32)
                nc.tensor.matmul(pt[:, 0:PC], brow[:],
                                 ident[0:PC, 0:PC], is_transpose=True)
                nc.scalar.copy(bkcol[:, i * PC:(i + 1) * PC], pt[:, 0:PC])

            # V-bias broadcast tiles
            bv1_bc = bias_small.tile([P, M], f32)
            dma.dma_start(out=bv1_bc[:], in_=bcast_ap(bv1_d))
            bv2_bc = bias_small.tile([P, M], f32)
            dma.dma_start(out=bv2_bc[:], in_=bcast_ap(bv2_d))

            def project(wd, CC, ct, wtpool, sbpool, bias_cols=None, bias_bc=None,
                        out_fm=None, out_tm=None):
                """Load W [M, CC*128], transpose to WT [c, m], then matmul.

                out_fm: feature-major out [p, mc, l] f32 (K-style), lhsT=WT rhs=CT.
                out_tm: token-major out [p, lc, m] bf16 (V-style), lhsT=CT rhs=WT.
                """
                wt = wtpool.tile([P, CC, M], bf16)
                load_T(wd, CC * P, wt, sbpool, ppool)
                for oc in range(PC):
                    for nn in range(2):
                        pt = pproj.tile([P, 512], f32)
                        for cc in range(CC):
                            if out_fm is not None:
                                lhs = wt[:, cc, oc * P:(oc + 1) * P]
                                rhs = ct[:, cc, nn * 512:(nn + 1) * 512]
                            else:
                                lhs = ct[:, cc, oc * P:(oc + 1) * P]
                                rhs = wt[:, cc, nn * 512:(nn + 1) * 512]
                            nc.tensor.matmul(pt[:], lhs, rhs,
                                             start=(cc == 0), stop=(cc == CC - 1))
                        if out_fm is not None:
                            nc.scalar.activation(
                                out_fm[:, oc, nn * 512:(nn + 1) * 512], pt[:],
                                IDENT, bias=bias_cols[:, oc:oc + 1])
                        else:
                            nc.vector.tensor_add(
                                out_tm[:, oc, nn * 512:(nn + 1) * 512], pt[:],
                                bias_bc[:, nn * 512:(nn + 1) * 512])

            with ExitStack() as p1:
                sb1 = p1.enter_context(tc.tile_pool(name="sb1", bufs=2))
                tp1 = p1.enter_context(tc.tile_pool(name="tp1", bufs=1))
                wtp1 = p1.enter_context(tc.tile_pool(name="wtp1", bufs=1))
                c1t = tp1.tile([P, D1 // P, L], bf16)  # [p, cc(4), l] 1MB
                load_T(c1_d, D1, c1t, sb1, ppool)
                project(wk1_d, D1 // P, c1t, wtp1, sb1,
                        bias_cols=bkcol[:, 0:PC], out_fm=k1t)
                project(wv1_d, D1 // P, c1t, wtp1, sb1,
                        bias_bc=bv1_bc, out_tm=v1)

            with ExitStack() as p2:
                sb2 = p2.enter_context(tc.tile_pool(name="sb2", bufs=2))
                tp2 = p2.enter_context(tc.tile_pool(name="tp2", bufs=1))
                wtp2 = p2.enter_context(tc.tile_pool(name="wtp2", bufs=1))
                c2t = tp2.tile([P, PC, L], bf16)       # 2MB
                load_T(c2_d, D2, c2t, sb2, ppool)
                project(wk2_d, D2 // P, c2t, wtp2, sb2,
                        bias_cols=bkcol[:, PC:2 * PC], out_fm=k2t)
                project(wv2_d, D2 // P, c2t, wtp2, sb2,
                        bias_bc=bv2_bc, out_tm=v2)
                # preload Wf12T here: phase A/B is PE-paced, ACT has slack
                wf12wt = wfw.tile([P, PC, M], bf16, tag="wfwt")
                load_T(wf12_d, M, wf12wt, sb2, ppool)
                wf12bb = wfb.tile([P, M], f32, tag="wfbb")
                dma.dma_start(out=wf12bb[:], in_=bcast_ap(bf12_d))

        # per-head K slices: head h -> partitions (h%2)*64.., chunk h//2
        def hd(t, h):
            return t[(h % 2) * DPH:(h % 2 + 1) * DPH, h // 2, :]

        # ---------- out projection helper ----------
        def out_project(wt, bbc, ctxT, out_d, ph):
            """Matmul part only; wt/bbc must be preloaded."""
            pout = ph.enter_context(tc.tile_pool(name="ps_o", bufs=2, space="PSUM"))
            opool = ph.enter_context(tc.tile_pool(name="ob", bufs=3))
            od = out_d.rearrange("(lc p) m -> p lc m", p=P)
            for lc in range(PC):
                for nn in range(2):
                    pt = pout.tile([P, 512], f32)
                    for cc in range(PC):
                        nc.tensor.matmul(
                            pt[:], ctxT[:, cc, lc * P:(lc + 1) * P],
                            wt[:, cc, nn * 512:(nn + 1) * 512],
                            start=(cc == 0), stop=(cc == PC - 1))
                    ot = opool.tile([P, 512], f32)
                    nc.vector.tensor_add(ot[:], pt[:],
                                         bbc[:, nn * 512:(nn + 1) * 512])
                    dma.dma_start(out=od[:, lc, nn * 512:(nn + 1) * 512], in_=ot[:])

        with ExitStack() as rest:
            apool = rest.enter_context(tc.tile_pool(name="attn", bufs=1))
            rq = apool.tile([P, PC, L], bf16)      # R[q,k] bf16: [p, qc, k] 2MB
            ctx12t = apool.tile([P, PC, L], bf16)  # CTX12T[m,q] 2MB

            # ---------- pass 1: k-major. ET, S2, V1', ctx12T, S1T ----------
            with ExitStack() as ph:
                s1pool = ph.enter_context(tc.tile_pool(name="s1", bufs=1))
                etpool = ph.enter_context(tc.tile_pool(name="et", bufs=2))
                psc = ph.enter_context(tc.tile_pool(name="ps_sc", bufs=3, space="PSUM"))
                pctx = ph.enter_context(tc.tile_pool(name="ps_cx", bufs=2, space="PSUM"))
                s1t = s1pool.tile([P, PC, L], f32)   # S1T[k,q]: [p, kc, q] 4MB

                for h in range(H):
                    k1h = hd(k1t, h)   # [64, 1024] (d, k)
                    k2h = hd(k2t, h)   # [64, 1024] (d, q)
                    et = etpool.tile([P, PC, L], bf16)   # ET[k,q] 2MB
                    # scoresT (2 matmuls fill a 2-bank PSUM tile) + exp + S2
                    for kc in range(PC):
                        pt = psc.tile([P, L], f32)
                        for qn in range(2):
                            nc.tensor.matmul(
                                pt[:, qn * 512:(qn + 1) * 512],
                                k1h[:, kc * P:(kc + 1) * P],
                                k2h[:, qn * 512:(qn + 1) * 512],
                                start=True, stop=True)
                        nc.scalar.activation(et[:, kc, :], pt[:], EXP,
                                             accum_out=s2[:, kc:kc + 1])
                    # invS2; V1_h *= invS2 (per-partition scale)
                    nc.vector.reciprocal_approx_fast(invs2[:], s2[:])
                    for kc in range(PC):
                        nc.vector.tensor_scalar_mul(
                            v1[:, kc, h * DPH:(h + 1) * DPH],
                            v1[:, kc, h * DPH:(h + 1) * DPH],
                            invs2[:, kc:kc + 1])
                    # ctx12T_h[d,q] = sum_k V1'_h[k,d] ET[k,q]
                    # (evac on ACT: DVE is the pass-1 pacer, ACT has slack)
                    for qn in range(2):
                        pt = pctx.tile([DPH, 512], f32)
                        for kc in range(PC):
                            nc.tensor.matmul(
                                pt[:], v1[:, kc, h * DPH:(h + 1) * DPH],
                                et[:, kc, qn * 512:(qn + 1) * 512],
                                start=(kc == 0), stop=(kc == PC - 1))
                        nc.scalar.copy(
                            hd(ctx12t, h)[:, qn * 512:(qn + 1) * 512], pt[:])
                    # S1T accumulation (fp32 += bf16), one fused op per head
                    if h == 0:
                        nc.vector.tensor_copy(s1t[:, :, :], et[:, :, :])
                    else:
                        nc.vector.tensor_add(s1t[:, :, :], s1t[:, :, :],
                                             et[:, :, :])

                # ---------- phase D: R[q,k] = transpose(1/S1T) ----------
                # split recip so first-half transposes start earlier
                nc.vector.reciprocal_approx_fast(s1t[:, 0:4, :], s1t[:, 0:4, :])
                nc.vector.reciprocal_approx_fast(s1t[:, 4:8, :], s1t[:, 4:8, :])
                transpose_1024(s1t, PC, rq, psc)

            # ctx21t allocated only after s1t is freed (SBUF pressure)
            c21pool = rest.enter_context(tc.tile_pool(name="c21", bufs=1))
            ctx21t = c21pool.tile([P, PC, L], bf16)  # CTX21T[m,k] 2MB

            # ---------- out12 projection (wt preloaded mid-pass-1) ----------
            with ExitStack() as ph:
                out_project(wf12wt, wf12bb, ctx12t, o12_d, ph)

            # ---------- pass 2: q-major. E, D=E*R, ctx21T ----------
            with ExitStack() as ph:
                epool = ph.enter_context(tc.tile_pool(name="e2", bufs=2))
                wfsb = ph.enter_context(tc.tile_pool(name="wfsb", bufs=1))
                psc = ph.enter_context(tc.tile_pool(name="ps_s2", bufs=2, space="PSUM"))
                pctx = ph.enter_context(tc.tile_pool(name="ps_c2", bufs=2, space="PSUM"))
                wfp2 = ph.enter_context(tc.tile_pool(name="ps_wf2", bufs=2, space="PSUM"))

                for h in range(H):
                    if h == 8:
                        wf21wt = wfw.tile([P, PC, M], bf16, tag="wfwt")
                        load_T(wf21_d, M, wf21wt, wfsb, wfp2,
                               evac=nc.vector.tensor_copy)
                        wf21bb = wfb.tile([P, M], f32, tag="wfbb")
                        dma.dma_start(out=wf21bb[:], in_=bcast_ap(bf21_d))
                    k1h = hd(k1t, h)
                    k2h = hd(k2t, h)
                    e = epool.tile([P, PC, L], bf16)   # E[q,k] 2MB
                    for qc in range(PC):
                        pt = psc.tile([P, L], f32)
                        for kn in range(2):
                            nc.tensor.matmul(
                                pt[:, kn * 512:(kn + 1) * 512],
                                k2h[:, qc * P:(qc + 1) * P],
                                k1h[:, kn * 512:(kn + 1) * 512],
                                start=True, stop=True)
                        nc.scalar.activation(e[:, qc, :], pt[:], EXP)
                        nc.vector.tensor_mul(e[:, qc, :], e[:, qc, :],
                                             rq[:, qc, :])
                    # ctx21T_h[d,k] = sum_q V2_h[q,d] D[q,k]
                    for kn in range(2):
                        pt = pctx.tile([DPH, 512], f32)
                        for qc in range(PC):
                            nc.tensor.matmul(
                                pt[:], v2[:, qc, h * DPH:(h + 1) * DPH],
                                e[:, qc, kn * 512:(kn + 1) * 512],
                                start=(qc == 0), stop=(qc == PC - 1))
                        nc.vector.tensor_copy(
                            hd(ctx21t, h)[:, kn * 512:(kn + 1) * 512], pt[:])

            # ---------- out21 projection ----------
            with ExitStack() as ph:
                out_project(wf21wt, wf21bb, ctx21t, o21_d, ph)

    nc.finalize()
    return nc


def _run(inputs, trace=False, tmpdir=None):
    from concourse import bass_utils
    if "nc" not in _CACHE:
        _CACHE["nc"] = _build()
    nc = _CACHE["nc"]

    shared = {
        "wk1": np.ascontiguousarray(inputs["Wk1"], np.float32),
        "wv1": np.ascontiguousarray(inputs["Wv1"], np.float32),
        "wk2": np.ascontiguousarray(inputs["Wk2"], np.float32),
        "wv2": np.ascontiguousarray(inputs["Wv2"], np.float32),
        "wf12": np.ascontiguousarray(inputs["Wf12"], np.float32),
        "wf21": np.ascontiguousarray(inputs["Wf21"], np.float32),
        "bk1": np.ascontiguousarray(inputs["bk1"], np.float32),
        "bv1": np.ascontiguousarray(inputs["bv1"], np.float32),
        "bk2": np.ascontiguousarray(inputs["bk2"], np.float32),
        "bv2": np.ascontiguousarray(inputs["bv2"], np.float32),
        "bf12": np.ascontiguousarray(inputs["bf12"], np.float32),
        "bf21": np.ascontiguousarray(inputs["bf21"], np.float32),
    }
    in_maps = []
    for b in range(B):
        m = dict(shared)
        m["ctx1"] = np.ascontiguousarray(inputs["ctx_1"][:, b, :], np.float32)
        m["ctx2"] = np.ascontiguousarray(inputs["ctx_2"][:, b, :], np.float32)
        in_maps.append(m)

    res = bass_utils.run_bass_kernel_spmd(
        nc, in_maps, core_ids=list(range(B)), trace=trace, tmpdir=tmpdir)
    out21 = np.stack([np.asarray(res.results[b]["out21"]) for b in range(B)],
                     axis=1).astype(np.float32)
    out12 = np.stack([np.asarray(res.results[b]["out12"]) for b in range(B)],
                     axis=1).astype(np.float32)
    return (out21, out12), res


def kernel(**inputs):
    outs, _ = _run(inputs, trace=False)
    return outs


# revision 49
# speedup vs baseline: 1.1487x; 1.0025x over previous
"""MultiHeadCoAttention Trainium2 kernel.

Math (per batch b, H=16 heads of d=64, L1=L2=1024, M=1024):
  K1 = ctx1 @ Wk1.T + bk1; V1 = ctx1 @ Wv1.T + bv1  (D1=512)
  K2 = ctx2 @ Wk2.T + bk2; V2 = ctx2 @ Wv2.T + bv2  (D2=1024)
  scores_h[q,k] = K2_h[q,:] . K1_h[k,:]
  dist12 = softmax over q (axis=2 in [B,H,q,k] -> per (h,k) over q)
  ctx12_h[q,:] = sum_k (E_h[q,k]/S2_h[k]) V1_h[k,:]   with E=exp(scores), S2_h[k]=sum_q E_h[q,k]
  dist21 = softmax over h: D_h[q,k] = E_h[q,k]/S1[q,k], S1[q,k]=sum_h E_h[q,k]
  ctx21_h[k,:] = sum_q D_h[q,k] V2_h[q,:]
  out12[q,:] = ctx12[q,:] @ Wf12.T + bf12 ; out21[k,:] = ctx21[k,:] @ Wf21.T + bf21

Restructured for HW: no max-subtraction (scores sigma~2.3, exp safe in fp32);
1/S2 folded into V1 rows; E stored bf16; S1 accumulated fp32 on DVE;
scores computed twice (k-major pass 1, q-major pass 2) on PE.
Sharding: data-parallel, one batch per core (8 cores).
"""

import numpy as np
from contextlib import ExitStack

L = 1024      # L1 == L2
D1 = 512
D2 = 1024
M = 1024
H = 16
DPH = 64
B = 8
P = 128
PC = L // P   # 8 partition-chunks per 1024-dim

_CACHE = {}


def _build():
    import concourse.bass as bass
    import concourse.bacc as bacc
    import concourse.mybir as mybir
    import concourse.tile as tile
    from concourse.masks import make_identity

    f32 = mybir.dt.float32
    f32r = mybir.dt.float32r
    bf16 = mybir.dt.bfloat16
    EXP = mybir.ActivationFunctionType.Exp
    IDENT = mybir.ActivationFunctionType.Identity

    nc = bacc.Bacc("TRN2")

    c1_d = nc.dram_tensor("ctx1", [L, D1], f32, kind="ExternalInput").ap()
    c2_d = nc.dram_tensor("ctx2", [L, D2], f32, kind="ExternalInput").ap()
    wk1_d = nc.dram_tensor("wk1", [M, D1], f32, kind="ExternalInput").ap()
    wv1_d = nc.dram_tensor("wv1", [M, D1], f32, kind="ExternalInput").ap()
    wk2_d = nc.dram_tensor("wk2", [M, D2], f32, kind="ExternalInput").ap()
    wv2_d = nc.dram_tensor("wv2", [M, D2], f32, kind="ExternalInput").ap()
    wf12_d = nc.dram_tensor("wf12", [M, M], f32, kind="ExternalInput").ap()
    wf21_d = nc.dram_tensor("wf21", [M, M], f32, kind="ExternalInput").ap()
    bk1_d = nc.dram_tensor("bk1", [M], f32, kind="ExternalInput").ap()
    bv1_d = nc.dram_tensor("bv1", [M], f32, kind="ExternalInput").ap()
    bk2_d = nc.dram_tensor("bk2", [M], f32, kind="ExternalInput").ap()
    bv2_d = nc.dram_tensor("bv2", [M], f32, kind="ExternalInput").ap()
    bf12_d = nc.dram_tensor("bf12", [M], f32, kind="ExternalInput").ap()
    bf21_d = nc.dram_tensor("bf21", [M], f32, kind="ExternalInput").ap()
    o21_d = nc.dram_tensor("out21", [L, M], f32, kind="ExternalOutput").ap()
    o12_d = nc.dram_tensor("out12", [L, M], f32, kind="ExternalOutput").ap()

    def bcast_ap(vec_ap, parts=P):
        return bass.AP(tensor=vec_ap.tensor, offset=vec_ap.offset,
                       ap=[[0, parts]] + list(vec_ap.ap))

    with tile.TileContext(nc) as tc, ExitStack() as top:
        dma = nc.default_dma_engine

        persist = top.enter_context(tc.tile_pool(name="persist", bufs=1))
        small = top.enter_context(tc.tile_pool(name="small", bufs=1))
        wfw = top.enter_context(tc.tile_pool(name="wfw", bufs=1))
        wfb = top.enter_context(tc.tile_pool(name="wfb", bufs=1))

        ident = small.tile([P, P], f32)
        make_identity(nc, ident[:])

        # ---- persistent SBUF tensors (whole kernel) ----
        k1t = persist.tile([P, PC, L], bf16)   # K1T[m,l]: [p, mc, l] 2MB
        k2t = persist.tile([P, PC, L], bf16)   # 2MB
        v1 = persist.tile([P, PC, M], bf16)    # V1 token-major [p, lc, m] 2MB
        v2 = persist.tile([P, PC, M], bf16)    # 2MB

        # K-biases as per-partition columns [p, mc]
        bkcol = small.tile([P, 2 * PC], f32)   # [:, 0:8]=bk1, [:, 8:16]=bk2
        s2 = small.tile([P, PC], f32)
        invs2 = small.tile([P, PC], f32)

        # ---------- helpers ----------
        def transpose_1024(src, src_cchunks, dst, psum_pool, dst_off=0,
                           evac=None):
            """src [P, AC, BC*128] -> dst [P, BC, dst_off + AC*128] transposed.

            src holds X[a, b] as [p, ac, b] with a=ac*128+p;
            dst receives XT[b, a] as [p, bc, dst_off + a].
            """
            if evac is None:
                evac = nc.scalar.copy
            AC = src_cchunks
            BC = src.shape[2] // P
            for bc in range(BC):
                for grp in range((AC + 3) // 4):
                    lo = grp * 4
                    hi = min(lo + 4, AC)
                    pt = psum_pool.tile([P, 512], f32)
                    for ac in range(lo, hi):
                        nc.tensor.matmul(
                            pt[:, (ac - lo) * P:(ac - lo + 1) * P],
                            src[:, ac, bc * P:(bc + 1) * P],
                            ident[:],
                            is_transpose=True,
                        )
                    evac(dst[:, bc, dst_off + lo * P:dst_off + hi * P],
                         pt[:, :(hi - lo) * P])

        def load_T(src_d, cols, dst, sbpool, ppool, evac=None):
            """DMA [1024, cols] DRAM tensor and PE-transpose into dst [P, cols//128, 1024]."""
            r = src_d.rearrange("(rc p) c -> p rc c", p=P)
            for ch in range(2):
                sb = sbpool.tile([P, 4, cols], f32)
                dma.dma_start(out=sb[:], in_=r[:, ch * 4:(ch + 1) * 4, :])
                transpose_1024(sb, 4, dst, ppool, dst_off=ch * 512, evac=evac)

        # ---------- phase A/B: loads, transposes, projections ----------
        with ExitStack() as ph:
            ppool = ph.enter_context(tc.tile_pool(name="ps_a", bufs=4, space="PSUM"))
            pproj = ph.enter_context(tc.tile_pool(name="ps_pj", bufs=2, space="PSUM"))
            bias_small = ph.enter_context(tc.tile_pool(name="bs", bufs=1))

            # K-bias columns: [1024] -> [8,128] sbuf -> PE transpose -> [128,8]
            for i, bd in enumerate((bk1_d, bk2_d)):
                brow = bias_small.tile([PC, P], f32)
                dma.dma_start(out=brow[:], in_=bd.rearrange("(a b) -> a b", a=PC))
                pt = ppool.tile([P, 512], f32)
                nc.tensor.matmul(pt[:, 0:PC], brow[:],
                                 ident[0:PC, 0:PC], is_transpose=True)
                nc.scalar.copy(bkcol[:, i * PC:(i + 1) * PC], pt[:, 0:PC])

            # V-bias broadcast tiles
            bv1_bc = bias_small.tile([P, M], f32)
            dma.dma_start(out=bv1_bc[:], in_=bcast_ap(bv1_d))
            bv2_bc = bias_small.tile([P, M], f32)
            dma.dma_start(out=bv2_bc[:], in_=bcast_ap(bv2_d))

            def project(wd, CC, ct, wtpool, sbpool, bias_cols=None, bias_bc=None,
                        out_fm=None, out_tm=None):
                """Load W [M, CC*128], transpose to WT [c, m], then matmul.

                out_fm: feature-major out [p, mc, l] f32 (K-style), lhsT=WT rhs=CT.
                out_tm: token-major out [p, lc, m] bf16 (V-style), lhsT=CT rhs=WT.
                """
                wt = wtpool.tile([P, CC, M], bf16)
                load_T(wd, CC * P, wt, sbpool, ppool)
                for oc in range(PC):
                    for nn in range(2):
                        pt = pproj.tile([P, 512], f32)
                        for cc in range(CC):
                            if out_fm is not None:
                                lhs = wt[:, cc, oc * P:(oc + 1) * P]
                                rhs = ct[:, cc, nn * 512:(nn + 1) * 512]
                            else:
                                lhs = ct[:, cc, oc * P:(oc + 1) * P]
                                rhs = wt[:, cc, nn * 512:(nn + 1) * 512]
                            nc.tensor.matmul(pt[:], lhs, rhs,
                                             start=(cc == 0), stop=(cc == CC - 1))
                        if out_fm is not None:
                            nc.scalar.activation(
                                out_fm[:, oc, nn * 512:(nn + 1) * 512], pt[:],
                                IDENT, bias=bias_cols[:, oc:oc + 1])
                        else:
                            nc.vector.tensor_add(
                                out_tm[:, oc, nn * 512:(nn + 1) * 512], pt[:],
                                bias_bc[:, nn * 512:(nn + 1) * 512])

            with ExitStack() as p1:
                sb1 = p1.enter_context(tc.tile_pool(name="sb1", bufs=2))
                tp1 = p1.enter_context(tc.tile_pool(name="tp1", bufs=1))
                wtp1 = p1.enter_context(tc.tile_pool(name="wtp1", bufs=1))
                c1t = tp1.tile([P, D1 // P, L], bf16)  # [p, cc(4), l] 1MB
                load_T(c1_d, D1, c1t, sb1, ppool)
                project(wk1_d, D1 // P, c1t, wtp1, sb1,
                        bias_cols=bkcol[:, 0:PC], out_fm=k1t)
                project(wv1_d, D1 // P, c1t, wtp1, sb1,
                        bias_bc=bv1_bc, out_tm=v1)

            with ExitStack() as p2:
                sb2 = p2.enter_context(tc.tile_pool(name="sb2", bufs=2))
                tp2 = p2.enter_context(tc.tile_pool(name="tp2", bufs=1))
                wtp2 = p2.enter_context(tc.tile_pool(name="wtp2", bufs=1))
                c2t = tp2.tile([P, PC, L], bf16)       # 2MB
                load_T(c2_d, D2, c2t, sb2, ppool)
                project(wk2_d, D2 // P, c2t, wtp2, sb2,
                        bias_cols=bkcol[:, PC:2 * PC], out_fm=k2t)
                project(wv2_d, D2 // P, c2t, wtp2, sb2,
                        bias_bc=bv2_bc, out_tm=v2)
                # preload Wf12T here: phase A/B is PE-paced, ACT has slack
                wf12wt = wfw.tile([P, PC, M], bf16, tag="wfwt")
                load_T(wf12_d, M, wf12wt, sb2, ppool)
                wf12bb = wfb.tile([P, M], f32, tag="wfbb")
                dma.dma_start(out=wf12bb[:], in_=bcast_ap(bf12_d))

        # per-head K slices: head h -> partitions (h%2)*64.., chunk h//2
        def hd(t, h):
            return t[(h % 2) * DPH:(h % 2 + 1) * DPH, h // 2, :]

        # ---------- out projection helper ----------
        def out_project(wt, bbc, ctxT, out_d, ph):
            """Matmul part only; wt/bbc must be preloaded."""
            pout = ph.enter_context(tc.tile_pool(name="ps_o", bufs=2, space="PSUM"))
            opool = ph.enter_context(tc.tile_pool(name="ob", bufs=3))
            od = out_d.rearrange("(lc p) m -> p lc m", p=P)
            for lc in range(PC):
                for nn in range(2):
                    pt = pout.tile([P, 512], f32)
                    for cc in range(PC):
                        nc.tensor.matmul(
                            pt[:], ctxT[:, cc, lc * P:(lc + 1) * P],
                            wt[:, cc, nn * 512:(nn + 1) * 512],
                            start=(cc == 0), stop=(cc == PC - 1))
                    ot = opool.tile([P, 512], f32)
                    nc.vector.tensor_add(ot[:], pt[:],
                                         bbc[:, nn * 512:(nn + 1) * 512])
                    dma.dma_start(out=od[:, lc, nn * 512:(nn + 1) * 512], in_=ot[:])

        with ExitStack() as rest:
            apool = rest.enter_context(tc.tile_pool(name="attn", bufs=1))
            rq = apool.tile([P, PC, L], bf16)      # R[q,k] bf16: [p, qc, k] 2MB
            ctx12t = apool.tile([P, PC, L], bf16)  # CTX12T[m,q] 2MB

            # ---------- pass 1: k-major. ET, S2, V1', ctx12T, S1T ----------
            with ExitStack() as ph:
                s1pool = ph.enter_context(tc.tile_pool(name="s1", bufs=1))
                etpool = ph.enter_context(tc.tile_pool(name="et", bufs=2))
                psc = ph.enter_context(tc.tile_pool(name="ps_sc", bufs=3, space="PSUM"))
                pctx = ph.enter_context(tc.tile_pool(name="ps_cx", bufs=2, space="PSUM"))
                s1t = s1pool.tile([P, PC, L], f32)   # S1T[k,q]: [p, kc, q] 4MB

                for h in range(H):
                    k1h = hd(k1t, h)   # [64, 1024] (d, k)
                    k2h = hd(k2t, h)   # [64, 1024] (d, q)
                    et = etpool.tile([P, PC, L], bf16)   # ET[k,q] 2MB
                    # scoresT (2 matmuls fill a 2-bank PSUM tile) + exp + S2
                    for kc in range(PC):
                        pt = psc.tile([P, L], f32)
                        for qn in range(2):
                            nc.tensor.matmul(
                                pt[:, qn * 512:(qn + 1) * 512],
                                k1h[:, kc * P:(kc + 1) * P],
                                k2h[:, qn * 512:(qn + 1) * 512],
                                start=True, stop=True)
                        nc.scalar.activation(et[:, kc, :], pt[:], EXP,
                                             accum_out=s2[:, kc:kc + 1])
                    # invS2; V1_h *= invS2 (per-partition scale)
                    nc.vector.reciprocal_approx_fast(invs2[:], s2[:])
                    for kc in range(PC):
                        nc.vector.tensor_scalar_mul(
                            v1[:, kc, h * DPH:(h + 1) * DPH],
                            v1[:, kc, h * DPH:(h + 1) * DPH],
                            invs2[:, kc:kc + 1])
                    # ctx12T_h[d,q] = sum_k V1'_h[k,d] ET[k,q]
                    for qn in range(2):
                        pt = pctx.tile([DPH, 512], f32)
                        for kc in range(PC):
                            nc.tensor.matmul(
                                pt[:], v1[:, kc, h * DPH:(h + 1) * DPH],
                                et[:, kc, qn * 512:(qn + 1) * 512],
                                start=(kc == 0), stop=(kc == PC - 1))
                        nc.vector.tensor_copy(
                            hd(ctx12t, h)[:, qn * 512:(qn + 1) * 512], pt[:])
                    # S1T accumulation (fp32 += bf16), one fused op per head
                    if h == 0:
                        nc.vector.tensor_copy(s1t[:, :, :], et[:, :, :])
                    else:
                        nc.vector.tensor_add(s1t[:, :, :], s1t[:, :, :],
                                             et[:, :, :])

                # ---------- phase D: R[q,k] = transpose(1/S1T) ----------
                # split recip so first-half transposes start earlier
                nc.vector.reciprocal_approx_fast(s1t[:, 0:4, :], s1t[:, 0:4, :])
                nc.vector.reciprocal_approx_fast(s1t[:, 4:8, :], s1t[:, 4:8, :])
                transpose_1024(s1t, PC, rq, psc)

            # ctx21t allocated only after s1t is freed (SBUF pressure)
            c21pool = rest.enter_context(tc.tile_pool(name="c21", bufs=1))
            ctx21t = c21pool.tile([P, PC, L], bf16)  # CTX21T[m,k] 2MB

            # ---------- out12 projection (wt preloaded mid-pass-1) ----------
            with ExitStack() as ph:
                out_project(wf12wt, wf12bb, ctx12t, o12_d, ph)

            # ---------- pass 2: q-major. E, D=E*R, ctx21T ----------
            with ExitStack() as ph:
                epool = ph.enter_context(tc.tile_pool(name="e2", bufs=2))
                wfsb = ph.enter_context(tc.tile_pool(name="wfsb", bufs=1))
                psc = ph.enter_context(tc.tile_pool(name="ps_s2", bufs=2, space="PSUM"))
                pctx = ph.enter_context(tc.tile_pool(name="ps_c2", bufs=2, space="PSUM"))
                wfp2 = ph.enter_context(tc.tile_pool(name="ps_wf2", bufs=2, space="PSUM"))

                for h in range(H):
                    if h == 8:
                        wf21wt = wfw.tile([P, PC, M], bf16, tag="wfwt")
                        load_T(wf21_d, M, wf21wt, wfsb, wfp2,
                               evac=nc.vector.tensor_copy)
                        wf21bb = wfb.tile([P, M], f32, tag="wfbb")
                        dma.dma_start(out=wf21bb[:], in_=bcast_ap(bf21_d))
                    k1h = hd(k1t, h)
                    k2h = hd(k2t, h)
                    e = epool.tile([P, PC, L], bf16)   # E[q,k] 2MB
                    for qc in range(PC):
                        pt = psc.tile([P, L], f32)
                        for kn in range(2):
                            nc.tensor.matmul(
                                pt[:, kn * 512:(kn + 1) * 512],
                                k2h[:, qc * P:(qc + 1) * P],
                                k1h[:, kn * 512:(kn + 1) * 512],
                                start=True, stop=True)
                        nc.scalar.activation(e[:, qc, :], pt[:], EXP)
                        nc.vector.tensor_mul(e[:, qc, :], e[:, qc, :],
                                             rq[:, qc, :])
                    # ctx21T_h[d,k] = sum_q V2_h[q,d] D[q,k]
                    for kn in range(2):
                        pt = pctx.tile([DPH, 512], f32)
                        for qc in range(PC):
                            nc.tensor.matmul(
                                pt[:], v2[:, qc, h * DPH:(h + 1) * DPH],
                                e[:, qc, kn * 512:(kn + 1) * 512],
                                start=(qc == 0), stop=(qc == PC - 1))
                        nc.vector.tensor_copy(
                            hd(ctx21t, h)[:, kn * 512:(kn + 1) * 512], pt[:])

            # ---------- out21 projection ----------
            with ExitStack() as ph:
                out_project(wf21wt, wf21bb, ctx21t, o21_d, ph)

    nc.finalize()
    return nc


def _run(inputs, trace=False, tmpdir=None):
    from concourse import bass_utils
    if "nc" not in _CACHE:
        _CACHE["nc"] = _build()
    nc = _CACHE["nc"]

    shared = {
        "wk1": np.ascontiguousarray(inputs["Wk1"], np.float32),
        "wv1": np.ascontiguousarray(inputs["Wv1"], np.float32),
        "wk2": np.ascontiguousarray(inputs["Wk2"], np.float32),
        "wv2": np.ascontiguousarray(inputs["Wv2"], np.float32),
        "wf12": np.ascontiguousarray(inputs["Wf12"], np.float32),
        "wf21": np.ascontiguousarray(inputs["Wf21"], np.float32),
        "bk1": np.ascontiguousarray(inputs["bk1"], np.float32),
        "bv1": np.ascontiguousarray(inputs["bv1"], np.float32),
        "bk2": np.ascontiguousarray(inputs["bk2"], np.float32),
        "bv2": np.ascontiguousarray(inputs["bv2"], np.float32),
        "bf12": np.ascontiguousarray(inputs["bf12"], np.float32),
        "bf21": np.ascontiguousarray(inputs["bf21"], np.float32),
    }
    in_maps = []
    for b in range(B):
        m = dict(shared)
        m["ctx1"] = np.ascontiguousarray(inputs["ctx_1"][:, b, :], np.float32)
        m["ctx2"] = np.ascontiguousarray(inputs["ctx_2"][:, b, :], np.float32)
        in_maps.append(m)

    res = bass_utils.run_bass_kernel_spmd(
        nc, in_maps, core_ids=list(range(B)), trace=trace, tmpdir=tmpdir)
    out21 = np.stack([np.asarray(res.results[b]["out21"]) for b in range(B)],
                     axis=1).astype(np.float32)
    out12 = np.stack([np.asarray(res.results[b]["out12"]) for b in range(B)],
                     axis=1).astype(np.float32)
    return (out21, out12), res


def kernel(**inputs):
    outs, _ = _run(inputs, trace=False)
    return outs
